# revision 1
# baseline (speedup 1.0000x reference)
"""Bass/Tile kernel for nn_MicrotubuleAttention on 8 Trainium2 NeuronCores.

Math: the reference adds (1 - gtp) * NEG (NEG = -1e9) to every causal
off-diagonal score. With gamma clipped to >= 1e-4, the smallest penalty is
-1e9 * (1 - exp(-1e-4)) ~= -1e5, so after float32 softmax (max-subtract +
exp) every off-diagonal weight underflows to exactly 0 and attention is
exactly the identity. Hence:

    out = repeat_gqa(x @ Wv) @ Wo = (x @ Wv) @ Wo_folded

where Wo_folded[c*64+d, :] = sum_r Wo[(4c+r)*64+d, :] sums the 4 query-head
row blocks that share KV head c. Q/K/RoPE/polarity/gamma provably do not
affect the f32 output.

Perf design (final, 46-48us measured vs 57.7us fp32 baseline):
- HBM descriptor runs >= 4KB for full rate (measured ~400GB/s aggregate
  at 4KB runs vs 85-130GB/s at 1-2KB): x and Wo load by full rows as
  large HWDGE fp32 DMAs split across BOTH issue engines (one HW-DGE ring
  alone serializes at ~300GB/s), dep-chained so ~2 transfers stay in
  flight in arrival-priority order; stores are full rows.
- Matmul inputs bf16 (gate 2e-2, lands ~3e-3). Wv loads via a SWDGE
  cast-DMA running from t~1 concurrently with x (slow path but fully off
  the critical chain); x casts ride the transpose copybacks; Wo casts
  ride the GQA fold adds.
- No partition-remap DMAs (v4 lost 10us to them): Wv columns are
  duplicated across both rr-halves in SBUF (8 small DVE copies), so
  stage 1 runs 4 g-groups whose PSUM output IS the replicated-vT layout
  stage 2 needs; the rr half of the GQA fold happens inside stage 2's
  partition contraction, the rp half in per-(g,h) DVE adds.
- One PSUM tag, 8 banks; stage 2 g-outer with all 8 (h, mi) tiles live
  so each Wo arrival -> fold -> matmul chain fires immediately.
- PE HAM warmup burst + keeper bursts dep-chained onto load completions.

Measured dead ends (do not retry without new evidence): SWDGE cast-DMAs
(~130GB/s/transfer); partition-remap SBUF DMAs for vT (serialize, ~10us);
half-partition [64,N] DVE fold adds (1.46us each, 2x slow); issuing Wo
dep-free at t0 (steals x bandwidth); moving Wo issues later to unblock
ACT copybacks (delays transfers more than it saves); alternating
stores onto the scalar ring (the store's issue-wait on a DVE copy
blocks ACT's remaining copybacks in FIFO order, +8us). PE runs saturated
21-36us, so stage interleaving cannot shorten it; remaining levers are
cutting the 4MB Wo read via cross-core AllGather of folded slices, or a
lower-precision rhs streaming mode.

Sharding: data parallel over rows. B*T = 4096 rows split 8 ways -> 512
rows per core; Wv/Wo broadcast.
"""

import os
import sys

import numpy as np

for _p in ("/opt/trn_rl_repo", "/opt/pypackages"):
    if os.path.isdir(_p) and _p not in sys.path:
        sys.path.append(_p)

B, T, D_MODEL = 2, 2048, 1024
H_Q, H_KV, D_HEAD = 16, 4, 64
N_CORES = 8
M_TOTAL = B * T              # 4096 rows
M_CORE = M_TOTAL // N_CORES  # 512 rows per core
P = 128
KK = D_MODEL // P            # 8 contraction chunks of 128
MC = M_CORE // P             # 4 row chunks of 128
NKV = H_KV * D_HEAD          # 256

TRACE = False          # test.py flips this to profile
TRACE_CORES = None
LAST_RESULTS = None    # BassKernelResults of the most recent run

_nc_cache = None


def _build_bass():
    import concourse.bass as bass
    import concourse.mybir as mybir
    import concourse.tile as tile
    from concourse import bacc
    from concourse.masks import make_identity
    from concourse.tile import add_dep_helper

    f32 = mybir.dt.float32
    bf16 = mybir.dt.bfloat16
    ts = bass.ts

    def dep(later, earlier, reason="order"):
        add_dep_helper(later.ins, earlier.ins, reason=reason)

    nc = bacc.Bacc(None)
    x_d = nc.declare_dram_parameter("x", [M_CORE, D_MODEL], f32, isOutput=False)
    wv_d = nc.declare_dram_parameter("wv", [P, KK, NKV], f32, isOutput=False)
    wo_d = nc.declare_dram_parameter("wo", [H_Q * D_HEAD, D_MODEL], f32, isOutput=False)
    out_d = nc.declare_dram_parameter("out", [M_CORE, D_MODEL], f32, isOutput=True)

    with tile.TileContext(nc) as tc:
        with (
            tc.tile_pool(name="const", bufs=1) as const,
            tc.tile_pool(name="x_pool", bufs=2) as x_pool,
            tc.tile_pool(name="wo_pool", bufs=4) as wo_pool,
            tc.tile_pool(name="o_pool", bufs=4) as o_pool,
            tc.tile_pool(name="psum", bufs=8, space="PSUM") as psum,
        ):
            ident_bf = const.tile([P, P], bf16)
            make_identity(nc, ident_bf)
            ident_f = const.tile([P, P], f32)
            make_identity(nc, ident_f)

            xT = const.tile([P, KK, M_CORE], bf16)     # [k_lo, kk, m]
            wv_bf = const.tile([P, KK, NKV], bf16)     # [k_lo, kk, j]
            # [k_lo, kk, g, (rr d)]: Wv cols of KV head g duplicated per rr
            wv_rep = const.tile([P, KK, 4, P], bf16)
            wof = const.tile([P, 4, D_MODEL], bf16)    # [(rr d), g, n]
            vT_rep = const.tile([P, 4, M_CORE], bf16)  # [(rr d), g, m]

            # ---- Wv: SWDGE cast-DMA from t~0, concurrent with x loads
            wv_dma = nc.gpsimd.dma_start(wv_bf[:], wv_d[:])

            # ---- PE warmup burst (bf16 single-pass MMs ~3.4us of activity)
            warm = psum.tile([P, P], f32, tag="ps")
            for _ in range(32):
                nc.tensor.matmul(warm[:], lhsT=ident_bf[:], rhs=ident_bf[:],
                                 start=True, stop=True)

            def keeper_burst(after, n=8):
                w = psum.tile([P, P], f32, tag="ps")
                first = nc.tensor.matmul(w[:], lhsT=ident_bf[:], rhs=ident_bf[:],
                                         start=True, stop=True)
                dep(first, after, "keep PE warm")
                for _ in range(n - 1):
                    nc.tensor.matmul(w[:], lhsT=ident_bf[:], rhs=ident_bf[:],
                                     start=True, stop=True)

            # ---- x + Wo: large HWDGE fp32 DMAs, full 4KB rows. Two issue
            # engines so both HW-DGE rings run (one engine's ring measured
            # ~300GB/s serial; sync+scalar pairs sustained ~400GB/s).
            # ACT-issued DMAs are emitted so their dep-waits resolve before
            # the next ACT compute op is ready (issue-wait blocks the FIFO).
            xv = x_d.rearrange("(half c p) k -> half p c k", half=2, c=2, p=P)
            x_tiles = []
            x_dmas = []
            for half in range(2):
                xf = x_pool.tile([P, 2, D_MODEL], f32, tag="x_in")
                x_tiles.append(xf)
                eng = nc.sync if half == 0 else nc.scalar
                x_dmas.append(eng.dma_start(xf[:], xv[half]))

            # Wo rows j = 256g + 128rp + q, q = (rr d); full 4KB rows per g
            wo_view = wo_d.rearrange("(g rp q) n -> g q rp n", g=4, rp=2, q=P)
            wo_tiles = []
            for _g in range(4):
                wo_t = wo_pool.tile([P, 2, D_MODEL], f32, tag="wo_raw")
                wo_tiles.append(wo_t)
            wo_dmas = {}
            wo_dmas[0] = nc.scalar.dma_start(wo_tiles[0][:], wo_view[0])
            wo_dmas[1] = nc.sync.dma_start(wo_tiles[1][:], wo_view[1])
            dep(wo_dmas[0], x_dmas[1])
            dep(wo_dmas[1], x_dmas[0])

            # ---- transposes: fp32 PE transpose, cast on ACT copyback -> xT
            xt_cbs = []
            for half in range(2):
                for c in range(2):
                    mi = 2 * half + c
                    for kh in range(2):
                        pt = psum.tile([P, 4, P], f32, tag="ps")
                        for j in range(4):
                            kk = 4 * kh + j
                            nc.tensor.transpose(
                                pt[:, j, :], x_tiles[half][:, c, ts(kk, P)],
                                ident_f[:])
                        xt_cbs.append(
                            nc.scalar.copy(xT[:, ts(kh, 4), ts(mi, P)], pt[:]))
                if half == 1:
                    keeper_burst(x_dmas[1])

            # remaining Wo loads, emitted after the xT copybacks so their
            # issue-waits on the ACT sequencer don't block ready copies
            wo_dmas[2] = nc.scalar.dma_start(wo_tiles[2][:], wo_view[2])
            wo_dmas[3] = nc.sync.dma_start(wo_tiles[3][:], wo_view[3])
            dep(wo_dmas[2], wo_dmas[0])
            dep(wo_dmas[3], wo_dmas[1])

            # ---- duplicate Wv cols across rr halves (8 small DVE copies)
            for g in range(4):
                for rr in range(2):
                    nc.vector.tensor_copy(
                        wv_rep[:, :, g, ts(rr, 64)],
                        wv_bf[:, :, bass.ds(64 * g, 64)])

            # ---- stage 1: vT_rep[(rr d), g, m] = v[m, 64g + d]. g-pairs
            # interleave across two PSUM banks so consecutive matmuls hit
            # different banks (MM-level ILP instead of a serial chain).
            for gp in range(2):
                ps1a = psum.tile([P, M_CORE], f32, tag="ps")
                ps1b = psum.tile([P, M_CORE], f32, tag="ps")
                for kk in range(KK):
                    for g, ps1 in ((2 * gp, ps1a), (2 * gp + 1, ps1b)):
                        nc.tensor.matmul(
                            ps1[:],
                            lhsT=wv_rep[:, kk, g, :],
                            rhs=xT[:, kk, :],
                            start=(kk == 0),
                            stop=(kk == KK - 1),
                        )
                nc.scalar.copy(vT_rep[:, 2 * gp, :], ps1a[:])
                nc.scalar.copy(vT_rep[:, 2 * gp + 1, :], ps1b[:])

            keeper_burst(wv_dma)

            # ---- GQA rp-fold per (g, h): wof[:, g, 512h+n], f32+f32 -> bf16
            for g in range(4):
                for h in range(2):
                    nc.vector.tensor_add(
                        wof[:, g, ts(h, 512)],
                        wo_tiles[g][:, 0, ts(h, 512)],
                        wo_tiles[g][:, 1, ts(h, 512)])

            keeper_burst(wo_dmas[1])

            # ---- stage 2 g-outer: 8 (h, mi) PSUM tiles live; each fold
            # immediately feeds its matmuls. Full-row stores per mi.
            pss = {}
            for mi in range(MC):
                for h in range(2):
                    ps2 = psum.tile([P, 512], f32, tag="ps")
                    pss[(h, mi)] = ps2
            for g in range(4):
                for mi in range(MC):
                    for h in range(2):
                        nc.tensor.matmul(
                            pss[(h, mi)][:],
                            lhsT=vT_rep[:, g, ts(mi, P)],
                            rhs=wof[:, g, ts(h, 512)],
                            start=(g == 0),
                            stop=(g == 3),
                        )
            for mi in range(MC):
                o_sb = o_pool.tile([P, 2, 512], f32, tag="o_sb")
                nc.scalar.copy(o_sb[:, 0, :], pss[(0, mi)][:])
                nc.vector.tensor_copy(o_sb[:, 1, :], pss[(1, mi)][:])
                nc.sync.dma_start(out_d[ts(mi, P), :], o_sb[:])

    nc.finalize()
    return nc


def _get_nc():
    global _nc_cache
    if _nc_cache is None:
        _nc_cache = _build_bass()
    return _nc_cache


def kernel(**inputs) -> np.ndarray:
    global LAST_RESULTS
    from concourse.bass_utils import run_bass_kernel_spmd

    x = np.ascontiguousarray(
        np.asarray(inputs["x"], dtype=np.float32).reshape(M_TOTAL, D_MODEL)
    )
    # Wv host layout permutation: wv2[p, kk, j] = Wv[128*kk + p, j] (pure
    # layout transform for DMA descriptor-run efficiency).
    wv = np.ascontiguousarray(
        np.asarray(inputs["Wv"], dtype=np.float32)
        .reshape(KK, P, NKV).transpose(1, 0, 2)
    )
    wo = np.ascontiguousarray(np.asarray(inputs["Wo"], dtype=np.float32))

    nc = _get_nc()
    in_maps = [
        {"x": x[i * M_CORE : (i + 1) * M_CORE], "wv": wv, "wo": wo}
        for i in range(N_CORES)
    ]
    res = run_bass_kernel_spmd(
        nc,
        in_maps,
        list(range(N_CORES)),
        trace=TRACE,
        trace_cores=TRACE_CORES,
    )
    LAST_RESULTS = res
    out = np.concatenate([r["out"] for r in res.results], axis=0)
    return out.reshape(B, T, D_MODEL)

